# revision 15
# baseline (speedup 1.0000x reference)
"""NT-Xent loss (SimCLR) on 8 Trainium2 NeuronCores — V3 (symmetric band).

Contract: kernel(z_i, z_j) -> np.float32 scalar loss matching the
reference NT-Xent (temperature 0.5).

The 8192x8192 matrix E = exp(2 zhat zhat^T) is symmetric, so only a
wrapped band needs computing.  Partition the rows into 16 blocks of
512.  Block b computes columns [b*512, b*512 + 4608) (9 chunks of 512,
wrapped mod 8192).  Per row x:
  A_x  = sum of the computed band of row x            (rowsums)
  C_x  = sum over columns x of the computed band rows, EXCLUDING each
         band's first and last 512-chunk              (colsums)
Then A_x covers column-blocks x..x+8, C_x covers x-7..x-1 (by symmetry
E[r,x] = E[x,r]), and since +8 == -8 (mod 16) every one of the 16
column-blocks is covered exactly once.  Row sum = A_x + C_x - E[x,x].

Per-core (core c owns rows [c*1024, (c+1)*1024) = blocks 2c, 2c+1):
  - Host rotates the fp8-quantized zhat^T columns by -c*1024, ships
    [128, 5120] (the union of both blocks' bands).  Host also computes
    positives, the diagonal terms and the final log/mean (O(N*D)).
  - PE: per (m, t) tile [128 rows, 1536 cols], 3 fp8 matmuls; plus
    accumulating ones-matmuls that produce the colsums C directly in
    PSUM (contraction over the 128 partitions).
  - exps split ACT (exp accum_out, writes E to SBUF bf16) / DVE
    (Schraudolph integer exp2 bits -> bf16, tensor_reduce rowsum).
  - Outputs: rowparts [128, 24] and the 8 colsum accumulators [1, 512].
"""

import os
import sys

if "/opt/trn_rl_repo" not in sys.path:
    sys.path.insert(0, "/opt/trn_rl_repo")

import numpy as np
import ml_dtypes

import concourse.bacc as bacc
import concourse.mybir as mybir
import concourse.tile as tile
from concourse.bass_utils import run_bass_kernel_spmd

B = 4096
D = 128
N = 2 * B  # 8192
CORES = 8
SLAB = N // CORES  # 1024 rows per core
MT = 8  # m-tiles of 128 rows
TW = 1536  # tile width
NT = 3  # tiles per m (band 4608 = 3*1536)
BAND = 4608
LOAD = 5120  # rotated columns loaded (union of both 512-blocks' bands)
EPS = 1e-12

f32 = mybir.dt.float32
bf16 = mybir.dt.bfloat16
fp8 = mybir.dt.float8e4
i16 = mybir.dt.int16

AF = mybir.ActivationFunctionType
OP = mybir.AluOpType

LOG2E = float(np.log2(np.e))
A1 = 2.0 * 128.0 * LOG2E
C_CORR = 0.0574 * 128.0 - 1.204
B1 = 127.0 * 128.0 - C_CORR

# DVE-consumer positions (t, m); 7 of 24, spread out, none at the tail
DVE_SET = {(0, 1), (0, 4), (0, 6), (1, 2), (1, 5), (2, 0), (2, 3)}


def _slot_contribs():
    """Emission-ordered list of colsum contributions per slot s (1..8).

    Tile (t, m) covers band-relative chunks g = 3t + k (k in 0..2).
    base = 0 for m<4 (block B0), 512 for m>=4 (B1).  Rotated chunk
    s = base/512 + g.  Colsums include only band-relative g in [1, 7].
    Returns {s: [(t, m, k), ...]} in emission order (t-major, m inner).
    """
    out = {s: [] for s in range(1, 9)}
    for t in range(NT):
        for m in range(MT):
            half = 0 if m < 4 else 1
            for k in range(3):
                g = 3 * t + k
                if 1 <= g <= 7:
                    s = half + g
                    out[s].append((t, m, k))
    return out


def build_nc():
    nc = bacc.Bacc("TRN2", target_bir_lowering=False, debug=False, num_devices=CORES)
    zT = nc.dram_tensor("zT", [128, LOAD], fp8, kind="ExternalInput").ap()
    out = nc.dram_tensor("out", [128, 26], f32, kind="ExternalOutput").ap()
    cout = nc.dram_tensor("cout", [8, 512], f32, kind="ExternalOutput").ap()

    contribs = _slot_contribs()
    done_slots = set()
    done_banks = set()
    first = {}
    last = {}
    for s, lst in contribs.items():
        first[s] = lst[0]
        last[s] = lst[-1]

    with tile.TileContext(nc) as tc:
        with (
            tc.tile_pool(name="big", bufs=1) as big,
            tc.tile_pool(name="ring", bufs=2, space="PSUM") as ring,
            tc.tile_pool(name="accp", bufs=1, space="PSUM") as accp,
            tc.tile_pool(name="ebp", bufs=3) as ebp,
            tc.tile_pool(name="wap", bufs=3) as wap,
        ):
            zt = big.tile([128, LOAD], fp8, tag="zt")
            rowparts = big.tile([128, 26], f32, tag="rowparts")
            csb = big.tile([128, 1024], f32, tag="csb")  # staged colsums
            ones = big.tile([128, 1], bf16, tag="ones")
            dummy = big.tile([128, 1], f32, tag="dummy")

            nc.sync.dma_start(zt[:, 0:512], zT[:, 0:512])
            nc.sync.dma_start(zt[:, 512:1024], zT[:, 512:1024])
            nc.sync.dma_start(zt[:, 1024:1536], zT[:, 1024:1536])
            nc.sync.dma_start(zt[:, 1536:3072], zT[:, 1536:3072])
            nc.sync.dma_start(zt[:, 3072:4608], zT[:, 3072:4608])
            nc.sync.dma_start(zt[:, 4608:5120], zT[:, 4608:5120])

            nc.vector.memset(ones[:], 1.0)
            nc.vector.memset(dummy[:], 0.0)
            nc.scalar.activation(dummy[:], dummy[:], AF.Exp, bias=0.0, scale=1.0)

            acc = accp.tile([128, 1024], f32, tag="acc")  # 2 banks, 8 slots

            def acc_view(s):
                i = s - 1
                boff = (i // 4) * 512
                p = 32 * (i % 4)
                return acc[p : p + 1, boff : boff + 512]

            def colsum(esb, t, m, k):
                half = 0 if m < 4 else 1
                g = 3 * t + k
                if not (1 <= g <= 7):
                    return
                s = half + g
                is_last = last[s] == (t, m, k)
                nc.tensor.matmul(
                    acc_view(s),
                    lhsT=ones[:],
                    rhs=esb[:, k * 512 : (k + 1) * 512],
                    start=(first[s] == (t, m, k)),
                    stop=is_last,
                    skip_group_check=True,
                    tile_position=(0, 32 * ((s - 1) % 4)),
                )
                if is_last:
                    done_slots.add(s)
                    for bank, slots in ((0, {1, 2, 3, 4}), (1, {5, 6, 7, 8})):
                        if slots <= done_slots and bank not in done_banks:
                            done_banks.add(bank)
                            boff = bank * 512
                            nc.vector.tensor_copy(
                                csb[0:97, boff : boff + 512],
                                acc[0:97, boff : boff + 512],
                            )
                            for s2 in sorted(slots):
                                p = 32 * ((s2 - 1) % 4)
                                nc.sync.dma_start(
                                    cout[s2 - 1 : s2, :],
                                    csb[p : p + 1, boff : boff + 512],
                                )

            # Software-pipelined emission: the colsum matmuls of tile
            # i-2 are emitted after tile i's production — their PE gate
            # (consumer i-2 done) then matches the ring-slot gate of
            # prod(i), so the in-order PE stream never adds stalls.
            pending = []  # [(esb, t, m), ...] colsums not yet emitted
            for t in range(NT):
                for m in range(MT):
                    base = 0 if m < 4 else 512
                    col0 = base + t * TW
                    ps = ring.tile([128, TW], f32, tag="mm")
                    for h in range(3):
                        nc.tensor.matmul(
                            ps[:, h * 512 : (h + 1) * 512],
                            lhsT=zt[:, m * 128 : (m + 1) * 128],
                            rhs=zt[:, col0 + h * 512 : col0 + (h + 1) * 512],
                            start=True,
                            stop=True,
                        )
                    if len(pending) >= 2:
                        pe, pt, pm = pending.pop(0)
                        for k in range(3):
                            colsum(pe, pt, pm, k)
                    rp = rowparts[:, t * MT + m : t * MT + m + 1]
                    if t == 0 and m == 0:
                        # first tile: consume per-512 as the DMA chunks
                        # land, so ACT starts ~1.5us earlier
                        wa = wap.tile([128, TW], bf16, tag="wa")
                        for h in range(3):
                            rph = rp if h == 0 else rowparts[:, 23 + h : 24 + h]
                            nc.scalar.activation(
                                wa[:, h * 512 : (h + 1) * 512],
                                ps[:, h * 512 : (h + 1) * 512],
                                AF.Exp,
                                bias=0.0,
                                scale=2.0,
                                accum_out=rph,
                            )
                        pending.append((wa[:], t, m))
                    elif (t, m) in DVE_SET:
                        eb = ebp.tile([128, TW], i16, tag="eb")
                        for h in range(3):
                            nc.vector.tensor_scalar(
                                eb[:, h * 512 : (h + 1) * 512],
                                ps[:, h * 512 : (h + 1) * 512],
                                A1,
                                B1,
                                OP.mult,
                                OP.add,
                            )
                        ebf = eb[:].bitcast(bf16)
                        nc.vector.tensor_reduce(
                            rp, ebf, axis=mybir.AxisListType.X, op=OP.add
                        )
                        pending.append((ebf, t, m))
                    else:
                        wa = wap.tile([128, TW], bf16, tag="wa")
                        nc.scalar.activation(
                            wa[:], ps[:], AF.Exp, bias=0.0, scale=2.0, accum_out=rp
                        )
                        pending.append((wa[:], t, m))
            for pe, pt, pm in pending:
                for k in range(3):
                    colsum(pe, pt, pm, k)

            nc.sync.dma_start(out[:], rowparts[:])

    nc.compile()
    return nc


_NC_CACHE = {}


def _get_nc():
    if "nc" not in _NC_CACHE:
        _NC_CACHE["nc"] = build_nc()
    return _NC_CACHE["nc"]


def kernel(z_i, z_j):
    z_i = np.asarray(z_i, dtype=np.float32)
    z_j = np.asarray(z_j, dtype=np.float32)
    z = np.concatenate([z_i, z_j], axis=0)  # [N, D]
    norm = np.sqrt((z * z).sum(axis=1, keepdims=True))
    zhat = z / np.maximum(norm, EPS)

    zq = zhat.astype(ml_dtypes.float8_e4m3)
    zqT = np.ascontiguousarray(zq.T)  # [128, 8192]

    in_maps = []
    for c in range(CORES):
        rot = np.roll(zqT, -c * SLAB, axis=1)[:, :LOAD]
        in_maps.append({"zT": np.ascontiguousarray(rot)})

    nc = _get_nc()
    kwargs = {}
    tdir = os.environ.get("NTX_TRACE_DIR")
    if tdir:
        kwargs = {"trace": True, "tmpdir": tdir, "trace_cores": [0]}
    res = run_bass_kernel_spmd(nc, in_maps, core_ids=list(range(CORES)), **kwargs)
    if tdir:
        _NC_CACHE["last_results"] = res

    # host epilogue in fp64
    A = np.zeros(N, dtype=np.float64)
    C = np.zeros(N, dtype=np.float64)
    for c in range(CORES):
        o = res.results[c]["out"].astype(np.float64)  # [128, 26]
        rs = o[:, 0:24].reshape(128, NT, MT).sum(axis=1)  # [p, m]
        for m in range(MT):
            r0 = c * SLAB + m * 128
            A[r0 : r0 + 128] += rs[:, m]
        A[c * SLAB : c * SLAB + 128] += o[:, 24] + o[:, 25]
        co = res.results[c]["cout"].astype(np.float64)  # [8, 512] slots 1..8
        for s in range(1, 9):
            j0 = (c * SLAB + s * 512) % N
            C[j0 : j0 + 512] += co[s - 1]

    zq64 = zq.astype(np.float64)
    diag = np.exp(2.0 * (zq64 * zq64).sum(axis=1))
    rowsums = A + C - diag
    lse = np.log(rowsums)

    zh64 = zhat.astype(np.float64)
    pos = 2.0 * np.concatenate(
        [
            (zh64[:B] * zh64[B:]).sum(axis=1),
            (zh64[B:] * zh64[:B]).sum(axis=1),
        ]
    )
    loss = (lse - pos).mean()
    return np.float32(loss)
